# revision 20
# baseline (speedup 1.0000x reference)
"""Trainium2 Bass kernel for nn_Conv2d_22222160789797.

Conv2d: x [32,128,56,56] f32, weight [256,128,3,3] (OIHW), stride 1, pad 1
-> out [32,256,56,56] f32.

Strategy: data-parallel over batch across 8 cores (4 images/core), bf16
operands, 1-D Winograd F(2,3) along W (host-side input/weight transform:
4 positions x 28 tiles per row instead of 3 kw taps x 56 cols), and the
3 kh taps accumulated in PSUM. PE work: 48 matmuls of free-dim 392 per
(img, oc-half) = 384 MMs/core ~ 63.4us of streaming at 2.4GHz.

Key structure (v2, from trace analysis of the v1 kernel at ~100us):

- Input layout is pos-major [img, ic, pos, padded_row, 28] in emission
  order [1,3,2,0], so every matmul's moving operand is one flat
  contiguous [392] slice, and image 0 streams in as 4 contiguous
  pos-slice DMAs (406KB each) -- the first real MM can start ~10.3us
  while the HAM clock-gate is still warming anyway.

- MM emission per (img, half) is (pos, kh)-outer / chunk-inner: the 4
  row-chunks' MMs share one LDWEIGHTS (12 weight loads per img-half
  instead of 48), keeping the PE at its ~166ns/MM streaming rate.

- PSUM: two [128, 1024] f32 tiles ("chunk pairs", 2 banks each) x
  bufs=2 = all 8 banks. Chunk c of a pair accumulates its 3 kh taps
  into the bank at offset c*512. Consumers read the pair through a
  [2, 392]-strided view, halving per-op fixed overhead (FD=784 ops
  instead of 2x FD=392: DVE ~960ns vs 1350ns).

- Output transform per pair, engine-legal (DVE one PSUM operand/op,
  ACT stages via activation-copy, GpSimd SBUF-only), pos order
  [1,3,2,0] so the odd-column path (c3, out1) completes mid-stream and
  only out0 = u + m0 (one DVE op) trails the final MM of an img-half:
    ACT:    c1 = m1, c3 = m3          (PSUM->SBUF stages, f32)
    DVE:    v = c1 - m2, u = c1 + m2
    GpSimd: out1(odd cols)  = v - c3  (bf16 out)
    DVE:    out0(even cols) = u + m0  (bf16 out)

- Output is written bf16 (halves the 12.8MB/core f32 output DMA; the
  host upcasts; bf16 rounding adds ~2e-3 rel err vs the 2e-2 gate).
  Output DMAs alternate between the sync and scalar HWDGE rings; input
  DMAs ride sync (weights + image-0 pos slices) and gpsimd (whole-image
  prefetch of images 1-3).
"""

import numpy as np

import concourse.tile as tile
from concourse import bacc, mybir
from concourse.bass_utils import run_bass_kernel_spmd

N_CORES = 8
B, IC, H, W = 32, 128, 56, 56
OC, KH, KW = 256, 3, 3
BPC = B // N_CORES          # images per core
PH = H + 2                  # padded rows
J = 28                      # winograd tiles per row (2 output cols each)
R = 14                      # output rows per chunk
N_CHUNKS = H // R           # 4
N_PAIRS = N_CHUNKS // 2     # 2 chunk-pairs per (img, half)
OC_HALVES = OC // 128       # 2
FD = R * J                  # 392 matmul free dim
BANK = 512                  # f32 elems per PSUM bank

# winograd position order as laid out on host / emitted on device
POS_SEQ = (1, 3, 2, 0)

_f32 = mybir.dt.float32
_bf16 = mybir.dt.bfloat16

_compiled_nc = None

N_WARMUP = 13


def _build(warmup=N_WARMUP):
    nc = bacc.Bacc("TRN2", target_bir_lowering=False, debug=False)
    # host-transformed input, pos-major: [img, ic, pos(4), padded row, J]
    x_d = nc.dram_tensor("x", [BPC, IC, 4, PH, J], _bf16,
                         kind="ExternalInput")
    # weights: [ic, (half, slice, posinslice, kh, 128)]
    w_d = nc.dram_tensor("w", [IC, OC_HALVES * 2 * 2 * KH * 128], _bf16,
                         kind="ExternalInput")
    o_d = nc.dram_tensor("out", [BPC, OC, H * W], _bf16,
                         kind="ExternalOutput")
    w6 = w_d[:].rearrange("p (h s q k c) -> p h s q k c",
                          h=OC_HALVES, s=2, q=2, k=KH, c=128)

    with tile.TileContext(nc) as tc:
        with (
            tc.tile_pool(name="w", bufs=1) as wpool,
            tc.tile_pool(name="x", bufs=1) as xpool,
            tc.tile_pool(name="o", bufs=4) as opool,
            tc.tile_pool(name="ps", bufs=8, space="PSUM") as pspool,
        ):
            if warmup:
                # Accumulating groups of 3 keep the warmup at the 327ns
                # cold streaming rate (isolated single MMs pay the ~180ns
                # drain each and only tick the HAM activity window at
                # ~60% busy).
                wscr = wpool.tile([128, 128], _bf16, name="wscr", tag="wscr")
                xscr = wpool.tile([128, FD], _bf16, name="xscr", tag="xscr")
                nc.gpsimd.memset(wscr[:], 0.0)
                nc.gpsimd.memset(xscr[:], 0.0)
                for gi, i in enumerate(range(0, warmup, 3)):
                    # alternate tags so a group's first MM never waits on
                    # the previous group's completion (bufs=1 WAR)
                    pwarm = pspool.tile([128, 2 * BANK], _f32, name="pwarm",
                                        tag=f"q{gi % 2}", bufs=1)
                    n = min(3, warmup - i)
                    for j in range(n):
                        nc.tensor.matmul(pwarm[:, 0:FD], wscr[:], xscr[:],
                                         start=(j == 0), stop=(j == n - 1))

            # weight tiles: per (half, slice) = 2 pos blocks x 3 kh x 128 oc
            whAB = []
            for half in range(OC_HALVES):
                a = wpool.tile([IC, 2, KH, 128], _bf16, name=f"wh{half}a",
                               tag=f"wh{half}a")
                bb = wpool.tile([IC, 2, KH, 128], _bf16, name=f"wh{half}b",
                                tag=f"wh{half}b")
                whAB.append((a, bb))

            # whole-image tiles, pos-major. ALL input DMAs ride the sync
            # ring in strict need-order: HWDGE drains one ring FIFO, so
            # queue position IS priority -- the image-1..3 prefetches sit
            # behind the critical image-0 pos slices + weight slices
            # instead of stealing SDMA packets from them (the v2 mistake:
            # xt prefetch on the gpsimd ring round-robined against the
            # head slices at packet granularity and delayed the first real
            # MM by ~10us).
            xt = [
                xpool.tile([IC, 4, PH, J], _bf16, name=f"xt{img}",
                           tag=f"xt{img}")
                for img in range(BPC)
            ]
            # image-0 pos slices split in row halves; units are ordered
            # (img, pair, half), so the first TWO units consume only the
            # a-halves (pair 0 = rows 0..29) and the b-halves have ~8us
            # to arrive. Weight slices are interleaved at need-order.
            RA = 2 * R + 2
            nc.sync.dma_start(whAB[0][0][:], w6[:, 0, 0])
            nc.sync.dma_start(xt[0][:, 0, 0:RA], x_d[0, :, 0, 0:RA])
            nc.sync.dma_start(xt[0][:, 1, 0:RA], x_d[0, :, 1, 0:RA])
            nc.sync.dma_start(whAB[0][1][:], w6[:, 0, 1])
            nc.sync.dma_start(xt[0][:, 2, 0:RA], x_d[0, :, 2, 0:RA])
            nc.sync.dma_start(whAB[1][0][:], w6[:, 1, 0])
            nc.sync.dma_start(xt[0][:, 3, 0:RA], x_d[0, :, 3, 0:RA])
            nc.sync.dma_start(whAB[1][1][:], w6[:, 1, 1])
            for q in range(4):
                nc.sync.dma_start(xt[0][:, q, 2 * R:PH],
                                  x_d[0, :, q, 2 * R:PH])
            for img in range(1, BPC):
                nc.sync.dma_start(xt[img][:], x_d[img])

            out_slot = 0

            def unit(img, half, pair, taper=False):
                """One scheduling unit: a single chunk-pair of an
                (img, oc-half). 4 pos blocks x 6 MMs (kh-outer, so each
                LDWEIGHTS covers 2 MMs); each pos block owns a fixed
                2-bank PSUM tile (tag q0..q3, bufs=1), so one unit's 4
                blocks occupy all 8 banks and a block's WAR dependency
                reaches back one full ~4us unit -- the previous unit's
                late consumers (v/u/out0 on DVE) never stall this unit's
                MMs, and no intra-unit block waits on a staging copy.

                `taper` (final unit): blocks run in order pos1, pos2,
                pos3, pos0 so v/u are computed two blocks early, the odd
                columns split per chunk (DVE takes chunk 0 straight from
                PSUM, GpSimd takes chunk 1 via a small staged c3), and
                per-chunk even combines + ~200KB DMAs on both rings
                leave only one FD=392 DVE op trailing the last matmul."""
                nonlocal out_slot
                pp = {}
                chunks = (2 * pair, 2 * pair + 1)
                osl = o_d[img, half * 128:half * 128 + 128, :]

                def pview(qi, n=2, lo=0):
                    return pp[qi].rearrange(
                        "p (b x) -> p b x", b=2, x=BANK)[:, lo:lo + n, 0:FD]

                def emit_block(qi):
                    p = pspool.tile([128, 2 * BANK], _f32, name=f"q{qi}",
                                    tag=f"q{qi}", bufs=1)
                    pp[qi] = p
                    s, qq = qi // 2, qi % 2
                    for kh in range(KH):
                        wt = whAB[half][s][:, qq, kh, :]
                        for ci, chunk in enumerate(chunks):
                            r0 = chunk * R + kh
                            nc.tensor.matmul(
                                p[:, ci * BANK:ci * BANK + FD],
                                wt,
                                xt[img][:, qi, r0:r0 + R, :],
                                start=(kh == 0),
                                stop=(kh == KH - 1),
                            )

                def stage_c1():
                    c1 = opool.tile([128, 2 * FD], _f32, name="c1",
                                    tag="c1", bufs=4)
                    nc.scalar.copy(c1[:], pview(0))
                    return c1

                def make_vu(c1):
                    v = opool.tile([128, 2 * FD], _f32, name="v", tag="v",
                                   bufs=4)
                    nc.vector.tensor_sub(v[:], c1[:], pview(2))
                    u = opool.tile([128, 2 * FD], _f32, name="u", tag="u",
                                   bufs=4)
                    nc.vector.tensor_add(u[:], c1[:], pview(2))
                    return v, u

                if not taper:
                    emit_block(0)            # pos 1
                    c1 = stage_c1()
                    emit_block(1)            # pos 3
                    c3 = opool.tile([128, 2 * FD], _f32, name="c3",
                                    tag="c3", bufs=4)
                    nc.scalar.copy(c3[:], pview(1))
                    emit_block(2)            # pos 2
                    ot = opool.tile([128, 4 * FD], _bf16, name="ot",
                                    tag="ot", bufs=10)
                    v, u = make_vu(c1)
                    nc.gpsimd.tensor_sub(ot[:, 1:4 * FD:2], v[:], c3[:])
                    emit_block(3)            # pos 0
                    nc.vector.tensor_add(ot[:, 0:4 * FD:2], u[:], pview(3))
                    if out_slot < 6:
                        ring = nc.scalar
                    else:
                        ring = nc.scalar if out_slot % 2 == 0 else nc.sync
                    n0 = chunks[0] * 2 * FD
                    ring.dma_start(osl[:, n0:n0 + 4 * FD], ot[:])
                    out_slot += 1
                else:
                    emit_block(0)            # pos 1
                    c1 = stage_c1()
                    emit_block(2)            # pos 2 (early!)
                    ot = opool.tile([128, 4 * FD], _bf16, name="ot",
                                    tag="ot", bufs=10)
                    v, u = make_vu(c1)
                    emit_block(1)            # pos 3
                    c3b = opool.tile([128, FD], _f32, name="c3b", tag="c3",
                                     bufs=4)
                    nc.scalar.copy(c3b[:], pview(1, n=1, lo=1))
                    nc.vector.tensor_sub(ot[:, 1:2 * FD:2], v[:, 0:FD],
                                         pview(1, n=1, lo=0))
                    nc.gpsimd.tensor_sub(ot[:, 2 * FD + 1:4 * FD:2],
                                         v[:, FD:2 * FD], c3b[:])
                    emit_block(3)            # pos 0
                    for ci, chunk in enumerate(chunks):
                        nc.vector.tensor_add(
                            ot[:, 2 * ci * FD:2 * (ci + 1) * FD:2],
                            u[:, ci * FD:(ci + 1) * FD],
                            pview(3, n=1, lo=ci))
                        ring = nc.scalar if ci == 0 else nc.sync
                        n0 = chunk * 2 * FD
                        ring.dma_start(osl[:, n0:n0 + 2 * FD],
                                       ot[:, 2 * ci * FD:2 * (ci + 1) * FD])

            for img in range(BPC):
                for pair in range(N_PAIRS):
                    for half in range(OC_HALVES):
                        last = (img == BPC - 1 and half == OC_HALVES - 1
                                and pair == N_PAIRS - 1)
                        unit(img, half, pair, taper=last)
    nc.compile()
    return nc


def _get_nc():
    global _compiled_nc
    if _compiled_nc is None:
        _compiled_nc = _build()
    return _compiled_nc


def _prep_inputs(x, weight):
    import ml_dtypes

    x = np.asarray(x, dtype=np.float32)
    weight = np.asarray(weight, dtype=np.float32)
    xp = np.zeros((B, IC, PH, PH), dtype=np.float32)
    xp[:, :, 1:H + 1, 1:W + 1] = x
    # host winograd input transform along W, pos-major in emission order
    # [1, 3, 2, 0]: [B, IC, 4, PH, J]
    d0 = xp[:, :, :, 0:2 * J - 1:2]
    d1 = xp[:, :, :, 1:2 * J:2]
    d2 = xp[:, :, :, 2:2 * J + 1:2]
    d3 = xp[:, :, :, 3:2 * J + 2:2]
    xt = np.empty((B, IC, 4, PH, J), dtype=ml_dtypes.bfloat16)
    xt[:, :, 0] = d1 + d2          # pos 1
    xt[:, :, 1] = d1 - d3          # pos 3
    xt[:, :, 2] = d2 - d1          # pos 2
    xt[:, :, 3] = d0 - d2          # pos 0
    # host winograd weight transform: Wt[ic, kh, pos, oc] = sum_kw G[pos,kw] w
    G = np.array(
        [[1, 0, 0], [0.5, 0.5, 0.5], [0.5, -0.5, 0.5], [0, 0, 1]],
        dtype=np.float32,
    )
    # -> [ic, half, slice, posinslice, kh, 128] with pos order [1, 3, 2, 0]
    wt = (
        np.einsum("pw,oihw->ihpo", G, weight)
        .reshape(IC, KH, 4, OC_HALVES, 128)[:, :, [1, 3, 2, 0]]
        .transpose(0, 3, 2, 1, 4)          # [ic, half, pos^, kh, 128]
        .reshape(IC, OC_HALVES, 2, 2, KH, 128)
        .reshape(IC, OC_HALVES * 2 * 2 * KH * 128)
        .astype(ml_dtypes.bfloat16)
    )
    in_maps = [
        {"x": np.ascontiguousarray(xt[c * BPC:(c + 1) * BPC]), "w": wt}
        for c in range(N_CORES)
    ]
    return in_maps


def _run(x, weight, trace=False):
    nc = _get_nc()
    in_maps = _prep_inputs(x, weight)
    res = run_bass_kernel_spmd(nc, in_maps, list(range(N_CORES)), trace=trace)
    out = np.concatenate(
        [np.asarray(res.results[c]["out"]) for c in range(N_CORES)], axis=0
    ).astype(np.float32).reshape(B, OC, H, W)
    return out, res


def kernel(x, weight):
    out, _ = _run(x, weight)
    return out
